# revision 39
# baseline (speedup 1.0000x reference)
"""PNAConv message-passing kernel for 8 TRN2 NeuronCores.

Strategy (node-sharded, degree-grouped, engine-balanced):
  - Nodes sorted by in-degree globally, dealt round-robin to 8 cores; each
    core owns all edges of its nodes (no collectives).
  - Per-edge work in [128 feat, edges] layout:
      h1 = Acomb-onehot + wb @ gather(x_src); m = w2 @ relu(h1).
    Odd-degree segments are padded to even width with one extra slot whose
    one-hot selects a -1e9 row, so relu gives m == 0 exactly and every
    sum/sumsq window halves cleanly.  min/max windows use overlapped halving
    over the true (unpadded) degree, so the pad slot never pollutes them.
  - Engine split per chunk: ACT relu + square; Pool min/max halvings (+DGE
    for the gather); DVE sum/sq halvings + the 4 final windowed reduces.
  - Aggregates kept bf16; every post-MLP matmul is bf16 (1 cyc/row on PE).
  - LayerNorm tail is computed per 512-column chunk, pipelined behind the
    edge loop; output leaves as [128, n_tot] (host transposes).
"""

import sys
import numpy as np

sys.path.insert(0, "/opt/trn_rl_repo")

import ml_dtypes

N, E, H, T, FIN, DEGBINS = 20000, 320000, 128, 4, 32, 128
NCORES = 8
CH = 1024          # edge slots per compute chunk
GIDX = 4096        # indices per dma_gather call
EPS_LN = 1e-5
ATILE = 58         # node columns per Acomb tile (+1 neg +5 bond = 64 rows)
NEGROW = 58
BONDROW = 59
NTOKP = 20096      # node table padded to 157*128

BF16 = ml_dtypes.bfloat16


# ----------------------------------------------------------------------------
# Planning: uniform per-core structure derived from the degree profile
# ----------------------------------------------------------------------------

class Plan:
    pass


def slotw(d):
    """Slot width of a degree-d segment (odd degrees pad by one)."""
    return d + (d & 1)


def make_plan(edge_index):
    src = np.asarray(edge_index[0]).astype(np.int64)
    dst = np.asarray(edge_index[1]).astype(np.int64)
    deg = np.bincount(dst, minlength=N)

    order = np.argsort(deg, kind="stable")
    core_nodes = [order[c::NCORES] for c in range(NCORES)]

    degrees = [int(d) for d in np.unique(deg)]
    n_d = {}
    for d in degrees:
        n_d[d] = max(int((deg[cn] == d).sum()) for cn in core_nodes)

    n_tot = sum(n_d.values())
    pad0 = (-n_tot) % 128
    if pad0:
        n_d[0] = n_d.get(0, 0) + pad0
        n_tot += pad0
    dlist = sorted(n_d.keys())

    col_deg = np.concatenate([np.full(n_d[d], d, np.int64) for d in dlist])
    assert len(col_deg) == n_tot and n_tot % 128 == 0
    col_D = col_deg + (col_deg & 1)

    # chunking (identical across cores); runs carry (col0, ncols, d, D, eoff)
    chunks = []
    c = 0
    while c < n_tot:
        e_used, c0 = 0, c
        while c < n_tot and e_used + col_D[c] <= CH:
            e_used += col_D[c]
            c += 1
        assert c > c0, "degree larger than chunk size"
        runs, eoff, rc = [], 0, c0
        while rc < c:
            d = int(col_deg[rc])
            rn = rc
            while rn < c and col_deg[rn] == d:
                rn += 1
            D = slotw(d)
            if d > 0:
                runs.append((int(rc), int(rn - rc), d, D, int(eoff)))
            eoff += (rn - rc) * D
            rc = rn
        assert c - c0 <= 256, "chunk column count exceeds hs tile width"
        chunks.append(dict(c0=int(c0), c1=int(c), nslot=int(e_used), runs=runs))
    while len(chunks) % (GIDX // CH):
        chunks.append(dict(c0=n_tot, c1=n_tot, nslot=0, runs=[]))
    nch = len(chunks)

    # A-tile matmul spans per chunk: (tile, j0, j1) over the chunk's CH slots
    for ch in chunks:
        spans, j = [], 0
        for col in range(ch["c0"], ch["c1"]):
            t, D = col // ATILE, int(col_D[col])
            if D == 0:
                continue
            if spans and spans[-1][0] == t:
                spans[-1][2] += D
            else:
                spans.append([t, j, j + D])
            j += D
        if spans:
            spans[-1][2] = CH          # cover tail pad slots (one-hot zero)
        ch["spans"] = [(int(a), int(b), int(cc)) for a, b, cc in spans]

    p = Plan()
    p.deg, p.core_nodes = deg, core_nodes
    p.n_d, p.dlist, p.n_tot = n_d, dlist, n_tot
    p.col_deg, p.col_D = col_deg, col_D
    p.chunks, p.nch = chunks, nch
    p.zero_cols = int(n_d.get(0, 0))
    p.ntiles = (n_tot + ATILE - 1) // ATILE

    # which gather groups are fully dead (trailing pad chunks)
    gpc = GIDX // CH
    p.group_live = [any(chunks[g * gpc + i]["runs"] for i in range(gpc))
                    for g in range(nch // gpc)]

    # per-core column -> global node id (-1 for dummy)
    p.col_node = []
    for cn in core_nodes:
        cols = np.full(n_tot, -1, np.int64)
        off = 0
        for d in dlist:
            mine = cn[deg[cn] == d]
            cols[off:off + len(mine)] = mine
            off += n_d[d]
        p.col_node.append(cols)

    eo = np.argsort(dst, kind="stable")
    starts = np.zeros(N + 1, np.int64)
    np.cumsum(np.bincount(dst, minlength=N), out=starts[1:])
    p.edge_order, p.edge_starts = eo, starts
    p.src = src
    return p


def pack_core_edges(p, c, src, bond):
    """Per-core padded edge arrays: src ids, bond ids, one-hot row kinds.

    arow: 0..ATILE-1 node row, NEGROW for in-run pad slots, -1 for dead slots.
    """
    EP = p.nch * CH
    srcv = np.zeros(EP, np.int64)
    bondv = np.full(EP, -1, np.int64)
    arow = np.full(EP, -1, np.int64)
    cols = p.col_node[c]
    eo, starts = p.edge_order, p.edge_starts
    for k, ch in enumerate(p.chunks):
        j = k * CH
        for col in range(ch["c0"], ch["c1"]):
            d = int(p.col_deg[col])
            D = int(p.col_D[col])
            if D == 0:
                continue
            node = cols[col]
            if node >= 0 and d > 0:
                eidx = eo[starts[node]:starts[node + 1]]
                srcv[j:j + d] = src[eidx]
                bondv[j:j + d] = bond[eidx]
                arow[j:j + d] = col % ATILE
            if D > d:
                arow[j + d:j + D] = NEGROW
            j += D
    return srcv, bondv, arow


def build_expdat(p, srcv, bondv, arow):
    EP = p.nch * CH
    exp = np.zeros((p.nch, 64, CH), BF16)
    k = np.arange(EP) // CH
    j = np.arange(EP) % CH
    real = (arow >= 0) & (arow < NEGROW)
    exp[k[real], arow[real], j[real]] = 1
    exp[k[real], BONDROW + bondv[real], j[real]] = 1
    padm = arow == NEGROW
    exp[k[padm], NEGROW, j[padm]] = 1
    return exp


def build_srcidx(p, srcv):
    """[ngrp, 128, GIDX//16]; one gather call per group of GIDX edges."""
    ngrp = (p.nch * CH) // GIDX
    out = np.zeros((ngrp, 128, GIDX // 16), np.int16)
    i = np.arange(GIDX)
    v = srcv.reshape(ngrp, GIDX).astype(np.int16)
    for rep in range(8):
        out[:, rep * 16 + (i % 16), i // 16] = v[:, i]
    return out


# ----------------------------------------------------------------------------
# Weight packing (host)
# ----------------------------------------------------------------------------

def blockdiag(ws):
    ws = np.asarray(ws)
    t, f, g = ws.shape
    out = np.zeros((t * f, t * g), np.float32)
    for i in range(t):
        out[i * f:(i + 1) * f, i * g:(i + 1) * g] = ws[i]
    return out


def pack_weights(ins, p):
    w = {}
    pre_w1 = np.asarray(ins["pre_w1"], np.float32)      # [T, 96, 32]
    w["wab"] = blockdiag(pre_w1[:, 0:FIN]).astype(BF16)
    w["wb"] = blockdiag(pre_w1[:, FIN:2 * FIN]).astype(BF16)
    table5 = (np.asarray(ins["bond_emb"], np.float32) @ np.asarray(ins["enc_w"], np.float32)
              + np.asarray(ins["enc_b"], np.float32))                       # [5, 32]
    wc = pre_w1[:, 2 * FIN:3 * FIN]                                        # [T, 32, 32]
    ctab = np.zeros((5, H), np.float32)
    b1 = np.asarray(ins["pre_b1"], np.float32)                             # [T, 32]
    for k in range(5):
        for t in range(T):
            ctab[k, t * FIN:(t + 1) * FIN] = table5[k] @ wc[t] + b1[t]
    ctab6 = np.concatenate([np.full((1, H), -1.0e9, np.float32), ctab])
    w["ctab6"] = np.ascontiguousarray(
        np.tile(ctab6.astype(BF16), (1, p.ntiles)))                         # [6, ntiles*128]
    w["w2"] = blockdiag(np.asarray(ins["pre_w2"], np.float32)).astype(BF16)

    post_w1 = np.asarray(ins["post_w1"], np.float32)                       # [T, 512, 32]
    blocks = []
    rowsel = [0] + [1 + a for a in range(5)] + [6 + a for a in range(5)] + [11 + a for a in range(5)]
    for r in rowsel:
        blocks.append(blockdiag(post_w1[:, 32 * r:32 * (r + 1)]))
    w["w1post"] = np.ascontiguousarray(
        np.concatenate(blocks, axis=1)).astype(BF16)                        # [128, 16*128]
    w["w2post"] = blockdiag(np.asarray(ins["post_w2"], np.float32)).astype(BF16)
    lin_w = np.asarray(ins["lin_w"], np.float32)
    w["wlin"] = lin_w.astype(BF16)
    lin_b = np.asarray(ins["lin_b"], np.float32)

    # small column pack: [onescol, wlinsum] as [128, 2] bf16
    small = np.zeros((128, 2), np.float32)
    small[:, 0] = 1.0
    small[:, 1] = lin_w.sum(axis=1)
    w["smallcols"] = small.astype(BF16)
    w["ones1"] = np.ones((1, 128), BF16)
    w["identb"] = np.eye(128, dtype=BF16)

    biases = np.zeros((128, 6), np.float32)
    biases[:, 0] = np.asarray(ins["post_b1"], np.float32).reshape(-1)
    biases[:, 1] = np.asarray(ins["post_b2"], np.float32).reshape(-1)
    biases[:, 2] = lin_b
    biases[:, 3] = np.asarray(ins["ln_g"], np.float32)
    biases[:, 4] = np.asarray(ins["ln_b"], np.float32)
    biases[:, 5] = -float(lin_b.sum()) / 128.0
    w["biases"] = biases

    # per-node scaler rows and the pre_b2 correction tensor
    mol_deg = np.asarray(ins["mol_deg"], np.float64)
    bins = np.arange(DEGBINS, dtype=np.float64)
    avg_log = float((np.log(bins + 1.0) * mol_deg).sum() / mol_deg.sum())

    d = p.col_deg.astype(np.float64)
    d1 = np.maximum(d, 1.0)
    logd = np.log(d1 + 1.0)
    invd = (1.0 / d1).astype(np.float32)
    amp = (logd / avg_log).astype(np.float32)
    att = (avg_log / logd).astype(np.float32)
    w["invd_bc"] = np.ascontiguousarray(np.broadcast_to(invd, (128, p.n_tot))).astype(BF16)
    w["amp_bc"] = np.ascontiguousarray(np.broadcast_to(amp, (128, p.n_tot))).astype(BF16)
    w["att_bc"] = np.ascontiguousarray(np.broadcast_to(att, (128, p.n_tot))).astype(BF16)

    b2 = np.asarray(ins["pre_b2"], np.float32).reshape(-1)                  # [128]
    # correction for reducing m without its bias; folded through post-MLP L1.
    WT = w["w1post"].astype(np.float32).reshape(128, 16, 128)
    def grp(i0):
        csum = WT[:, i0, :].T @ b2
        cmmm = sum(WT[:, i0 + a, :].T @ b2 for a in (1, 2, 3))
        return csum, cmmm
    cs_id, cm_id = grp(1)
    cs_am, cm_am = grp(6)
    cs_at, cm_at = grp(11)
    dcol = d.astype(np.float32)
    corr = (cm_id[:, None] + np.outer(cs_id, dcol)
            + amp[None, :] * (cm_am[:, None] + np.outer(cs_am, dcol))
            + att[None, :] * (cm_at[:, None] + np.outer(cs_at, dcol)))
    w["corr"] = np.ascontiguousarray(corr).astype(BF16)
    return w


# ----------------------------------------------------------------------------
# Device program
# ----------------------------------------------------------------------------

def build_program(p):
    import concourse.bass as bass
    import concourse.tile as tile
    from concourse import bacc, mybir

    f32 = mybir.dt.float32
    bf16 = mybir.dt.bfloat16
    i16 = mybir.dt.int16
    AX = mybir.AxisListType.X
    OP = mybir.AluOpType
    AF = mybir.ActivationFunctionType

    n_tot, nch, ntiles = p.n_tot, p.nch, p.ntiles

    nc = bacc.Bacc("TRN2", target_bir_lowering=False, debug=False)

    d_exp = nc.dram_tensor("expdat", [nch, 64, CH], bf16, kind="ExternalInput")
    ngrp = (nch * CH) // GIDX
    d_idx = nc.dram_tensor("srcidx", [ngrp, 128, GIDX // 16], i16,
                           kind="ExternalInput")
    d_xbf = nc.dram_tensor("xbf", [128, n_tot], bf16, kind="ExternalInput")
    d_xfull = nc.dram_tensor("xfull", [128, NTOKP], bf16, kind="ExternalInput")
    d_wab = nc.dram_tensor("wab", [128, 128], bf16, kind="ExternalInput")
    d_wb = nc.dram_tensor("wb", [128, 128], bf16, kind="ExternalInput")
    d_ctab6 = nc.dram_tensor("ctab6", [6, ntiles * 128], bf16, kind="ExternalInput")
    d_w2 = nc.dram_tensor("w2", [128, 128], bf16, kind="ExternalInput")
    d_w1post = nc.dram_tensor("w1post", [128, 16 * 128], bf16, kind="ExternalInput")
    d_w2post = nc.dram_tensor("w2post", [128, 128], bf16, kind="ExternalInput")
    d_wlin = nc.dram_tensor("wlin", [128, 128], bf16, kind="ExternalInput")
    d_small = nc.dram_tensor("smallcols", [128, 2], bf16, kind="ExternalInput")
    d_ones1 = nc.dram_tensor("ones1", [1, 128], bf16, kind="ExternalInput")
    d_identb = nc.dram_tensor("identb", [128, 128], bf16, kind="ExternalInput")
    d_bias = nc.dram_tensor("biases", [128, 6], f32, kind="ExternalInput")
    d_invd = nc.dram_tensor("invd_bc", [128, n_tot], bf16, kind="ExternalInput")
    d_amp = nc.dram_tensor("amp_bc", [128, n_tot], bf16, kind="ExternalInput")
    d_att = nc.dram_tensor("att_bc", [128, n_tot], bf16, kind="ExternalInput")
    d_corr = nc.dram_tensor("corr", [128, n_tot], bf16, kind="ExternalInput")
    d_out = nc.dram_tensor("out", [128, n_tot], f32, kind="ExternalOutput")

    # col-chunk cc (512 cols) is ready after the last edge chunk touching it
    ready = {}
    for cc in range(n_tot // 512):
        last = 0
        for k2, ch2 in enumerate(p.chunks):
            if ch2["c0"] < 512 * (cc + 1) and ch2["runs"]:
                last = k2
        ready.setdefault(last, []).append(cc)
    first_ready = min(ready.keys())

    with nc.allow_low_precision(reason="bf16 aggregates within rel-err budget"):
      with tile.TileContext(nc) as tc:
        with tc.tile_pool(name="persist", bufs=1) as per:
            # --- persistent SBUF tensors; critical loads first ---------------
            xbf = per.tile([128, n_tot], bf16)
            nc.sync.dma_start(xbf[:], d_xbf[:])
            wab = per.tile([128, 128], bf16); nc.sync.dma_start(wab[:], d_wab[:])
            wb = per.tile([128, 128], bf16); nc.sync.dma_start(wb[:], d_wb[:])
            w2 = per.tile([128, 128], bf16); nc.sync.dma_start(w2[:], d_w2[:])
            identb = per.tile([128, 128], bf16)
            nc.sync.dma_start(identb[:], d_identb[:])
            xtok = per.tile([128, NTOKP], bf16)
            nc.sync.dma_start(xtok[:], d_xfull[:])

            acomb = per.tile([64, ntiles, 128], bf16)
            w1post = per.tile([128, 16, 128], bf16)
            w2post = per.tile([128, 128], bf16)
            wlin = per.tile([128, 128], bf16)
            smallcols = per.tile([128, 2], bf16)
            ones1 = per.tile([1, 128], bf16)
            biases = per.tile([128, 6], f32)
            invd_bc = per.tile([128, n_tot], bf16)
            amp_bc = per.tile([128, n_tot], bf16)
            att_bc = per.tile([128, n_tot], bf16)
            corr = per.tile([128, n_tot], bf16)
            eps_col = per.tile([128, 1], f32)

            def load_stage(st):
                if st == 0:
                    nc.sync.dma_start(w1post[:].rearrange("p a b -> p (a b)"),
                                      d_w1post[:])
                    nc.sync.dma_start(biases[:], d_bias[:])
                    nc.gpsimd.memset(eps_col[:], EPS_LN)
                elif st == 1:
                    nc.sync.dma_start(w2post[:], d_w2post[:])
                    nc.sync.dma_start(wlin[:], d_wlin[:])
                    nc.sync.dma_start(smallcols[:], d_small[:])
                    nc.sync.dma_start(ones1[:], d_ones1[:])
            nstage = 2

            # aggregate buffers (bf16)
            s_ss = per.tile([128, 2, n_tot], bf16)
            s_min = per.tile([128, n_tot], bf16)
            s_max = per.tile([128, n_tot], bf16)

            # --- setup: Acomb (bf16 matmuls; copies on Pool) -----------------
            nc.gpsimd.memset(acomb[:], 0.0)
            nc.sync.dma_start(
                acomb[NEGROW:64, :, :],
                d_ctab6[:].rearrange("p (t f) -> p t f", f=128))
            with tc.tile_pool(name="su_ps", bufs=2, space="PSUM") as sps:
                for t in range(ntiles):
                    nt = min(ATILE, n_tot - t * ATILE)
                    aps = sps.tile([64, 128], f32, tag="aps")
                    nc.tensor.matmul(aps[0:nt, :], xbf[:, t * ATILE:t * ATILE + nt],
                                     wab[:], start=True, stop=True)
                    nc.vector.tensor_scalar_mul(acomb[0:nt, t, :],
                                                aps[0:nt, :], 1.0)

            if p.zero_cols:
                z = p.zero_cols
                nc.gpsimd.memset(s_ss[:, :, 0:z], 0.0)
                nc.gpsimd.memset(s_min[:, 0:z], 0.0)
                nc.gpsimd.memset(s_max[:, 0:z], 0.0)

            # --- edge phase with interleaved post passes ---------------------
            with (
                tc.tile_pool(name="eg_ps", bufs=1, space="PSUM") as pps,
                tc.tile_pool(name="eg_hs", bufs=1, space="PSUM") as phs,
                tc.tile_pool(name="pa_uid", bufs=1, space="PSUM") as puid,
                tc.tile_pool(name="pa_acc", bufs=1, space="PSUM") as pacc,
                tc.tile_pool(name="eg_sb", bufs=2) as esb,
                tc.tile_pool(name="eg_gt", bufs=2) as egt,
                tc.tile_pool(name="pa_sb", bufs=2) as pasb,
            ):
                # stage s of col-chunk cc issues at chunk ready[cc]+s; a
                # 5-deep pipeline so no engine convoys behind a full passA.
                # stage 0 = scaler-table slice prefetch (can fire early).
                stage_plan = {}
                for k0, ccs in ready.items():
                    for cc in ccs:
                        stage_plan.setdefault(max(0, k0 - 1), []).append((0, cc))
                        for s in range(1, 5):
                            stage_plan.setdefault(min(k0 + s, nch - 1),
                                                  []).append((s, cc))
                # re-sort so stages issue in (chunk, stage) order
                stage_plan = {k2: sorted(v) for k2, v in stage_plan.items()}
                cst = {}

                def pa_stage(stg, cc):
                    a, b = 512 * cc, 512 * (cc + 1)
                    s_sum = s_ss[:, 0, a:b]
                    s_sumsq = s_ss[:, 1, a:b]
                    st = cst.setdefault(cc, {})
                    if stg == 0:
                        # prefetch this block's scaler-table slices
                        nc.gpsimd.dma_start(invd_bc[:, a:b], d_invd[:, a:b])
                        nc.gpsimd.dma_start(corr[:, a:b], d_corr[:, a:b])
                        nc.gpsimd.dma_start(amp_bc[:, a:b], d_amp[:, a:b])
                        nc.gpsimd.dma_start(att_bc[:, a:b], d_att[:, a:b])
                    elif stg == 1:
                        # statistics (all bf16)
                        s_mean = pasb.tile([128, 512], bf16, tag="tmean")
                        nc.vector.tensor_tensor(s_mean[:], s_sum,
                                                invd_bc[:, a:b], op=OP.mult)
                        m2 = pasb.tile([128, 512], bf16, tag="tb")
                        nc.vector.tensor_tensor(m2[:], s_mean[:],
                                                s_mean[:], op=OP.mult)
                        var = pasb.tile([128, 512], bf16, tag="ta")
                        nc.vector.tensor_tensor(var[:], s_sumsq,
                                                invd_bc[:, a:b], op=OP.mult)
                        nc.vector.tensor_tensor(var[:], var[:], m2[:],
                                                op=OP.subtract)
                        vclamp = pasb.tile([128, 512], bf16, tag="tvc")
                        nc.vector.tensor_scalar_max(vclamp[:], var[:], 0.0)
                        tstd = pasb.tile([128, 512], bf16, tag="tstd")
                        nc.scalar.activation(tstd[:], vclamp[:], AF.Sqrt,
                                             bias=eps_col[:])
                        st["s_mean"], st["tstd"] = s_mean, tstd
                    elif stg == 2:
                        s_mean, tstd = st["s_mean"], st["tstd"]
                        uid = puid.tile([128, 512], f32, tag="uid")
                        nc.tensor.matmul(uid[:], w1post[:, 0, :], xbf[:, a:b],
                                         start=True, stop=False)
                        nc.tensor.matmul(uid[:], w1post[:, 1, :], s_sum,
                                         start=False, stop=False)
                        nc.tensor.matmul(uid[:], w1post[:, 2, :], s_mean[:],
                                         start=False, stop=False)
                        nc.tensor.matmul(uid[:], w1post[:, 3, :], s_min[:, a:b],
                                         start=False, stop=False)
                        nc.tensor.matmul(uid[:], w1post[:, 4, :], s_max[:, a:b],
                                         start=False, stop=False)
                        nc.tensor.matmul(uid[:], w1post[:, 5, :], tstd[:],
                                         start=False, stop=False)
                        nc.tensor.matmul(uid[:], identb[:], corr[:, a:b],
                                         start=False, stop=False)
                        uam = pacc.tile([128, 512], f32, tag="uacc")
                        nc.tensor.matmul(uam[:], w1post[:, 6, :], s_sum,
                                         start=True, stop=False)
                        nc.tensor.matmul(uam[:], w1post[:, 7, :], s_mean[:],
                                         start=False, stop=False)
                        nc.tensor.matmul(uam[:], w1post[:, 8, :], s_min[:, a:b],
                                         start=False, stop=False)
                        nc.tensor.matmul(uam[:], w1post[:, 9, :], s_max[:, a:b],
                                         start=False, stop=False)
                        nc.tensor.matmul(uam[:], w1post[:, 10, :], tstd[:],
                                         start=False, stop=True)
                        st["uid"], st["uam"] = uid, uam
                    elif stg == 3:
                        s_mean, tstd, uid = st["s_mean"], st["tstd"], st["uid"]
                        t1 = pasb.tile([128, 512], bf16, tag="ta")
                        nc.vector.tensor_tensor(t1[:], st["uam"][:],
                                                amp_bc[:, a:b], op=OP.mult)
                        nc.tensor.matmul(uid[:], identb[:], t1[:],
                                         start=False, stop=False)
                        uat = pacc.tile([128, 512], f32, tag="uacc")
                        nc.tensor.matmul(uat[:], w1post[:, 11, :], s_sum,
                                         start=True, stop=False)
                        nc.tensor.matmul(uat[:], w1post[:, 12, :], s_mean[:],
                                         start=False, stop=False)
                        nc.tensor.matmul(uat[:], w1post[:, 13, :], s_min[:, a:b],
                                         start=False, stop=False)
                        nc.tensor.matmul(uat[:], w1post[:, 14, :], s_max[:, a:b],
                                         start=False, stop=False)
                        nc.tensor.matmul(uat[:], w1post[:, 15, :], tstd[:],
                                         start=False, stop=True)
                        t2 = pasb.tile([128, 512], bf16, tag="tb")
                        nc.vector.tensor_tensor(t2[:], uat[:], att_bc[:, a:b],
                                                op=OP.mult)
                        nc.tensor.matmul(uid[:], identb[:], t2[:],
                                         start=False, stop=True)
                        r1 = pasb.tile([128, 512], bf16, tag="tr")
                        nc.scalar.activation(r1[:], uid[:], AF.Relu,
                                             bias=biases[:, 0:1])
                        u2 = pacc.tile([128, 512], f32, tag="uacc")
                        nc.tensor.matmul(u2[:], w2post[:], r1[:],
                                         start=True, stop=True)
                        s2 = pasb.tile([128, 512], bf16, tag="ta")
                        nc.scalar.activation(s2[:], u2[:], AF.Identity,
                                             bias=biases[:, 1:2])
                        st["s2"] = s2
                    else:
                        s2 = st["s2"]
                        # final linear with the LN mean-subtract folded in
                        lp = pacc.tile([128, 512], f32, tag="uacc")
                        nc.tensor.matmul(lp[:], wlin[:], s2[:],
                                         start=True, stop=False)
                        mu_p = puid.tile([1, 512], f32, tag="uid")
                        nc.tensor.matmul(mu_p[:], smallcols[:, 1:2], s2[:],
                                         start=True, stop=True)
                        muN = pasb.tile([1, 512], bf16, tag="muN")
                        nc.scalar.activation(muN[:], mu_p[:], AF.Identity,
                                             scale=-1.0 / 128.0,
                                             bias=biases[0:1, 5:6])
                        nc.tensor.matmul(lp[:], ones1[:], muN[:],
                                         start=False, stop=True)
                        lbuf = pasb.tile([128, 512], bf16, tag="tb")
                        nc.scalar.activation(lbuf[:], lp[:], AF.Identity,
                                             bias=biases[:, 2:3])
                        lsq = pasb.tile([128, 512], bf16, tag="tvc")
                        nc.vector.tensor_tensor(lsq[:], lbuf[:],
                                                lbuf[:], op=OP.mult)
                        sq_p = puid.tile([1, 512], f32, tag="uid")
                        nc.tensor.matmul(sq_p[:], smallcols[:, 0:1], lsq[:],
                                         start=True, stop=True)
                        rsd = pasb.tile([1, 512], f32, tag="rsd")
                        nc.scalar.activation(rsd[:], sq_p[:], AF.Sqrt,
                                             scale=1.0 / 128.0,
                                             bias=eps_col[0:1, :])
                        rs = pasb.tile([1, 512], bf16, tag="muN")
                        nc.vector.reciprocal(rs[:], rsd[:])
                        rsb = pacc.tile([128, 512], f32, tag="uacc")
                        nc.tensor.matmul(rsb[:], ones1[:], rs[:],
                                         start=True, stop=True)
                        tno = pasb.tile([128, 512], bf16, tag="ta")
                        nc.vector.tensor_tensor(tno[:], lbuf[:], rsb[:],
                                                op=OP.mult)
                        fin = pasb.tile([128, 512], bf16, tag="tr")
                        nc.scalar.activation(fin[:], tno[:], AF.Relu,
                                             bias=biases[:, 4:5],
                                             scale=biases[:, 3:4])
                        outb = pasb.tile([128, 512], f32, tag="outb")
                        nc.vector.tensor_tensor(outb[:], fin[:],
                                                xbf[:, a:b], op=OP.add)
                        nc.gpsimd.dma_start(d_out[:, a:b], outb[:])
                        cst.pop(cc, None)

                for k in range(nch):
                    kk = k % (GIDX // CH)
                    g = k // (GIDX // CH)
                    if kk == 0 and p.group_live[g]:
                        gidx = egt.tile([128, GIDX // 16], i16, tag="gidx")
                        nc.sync.dma_start(gidx[:], d_idx[g, :, :])
                        bg = egt.tile([128, 1, GIDX], bf16, tag="bg")
                        nc.gpsimd.dma_gather(
                            bg[:], xtok[:], gidx[:], GIDX, GIDX, 128,
                            transpose=True, sbuf_tokens_per_rank=128,
                            sbuf_free_dim_per_rank=256, single_packet=False,
                        )
                    ch = p.chunks[k]
                    if ch["runs"]:
                        expt = esb.tile([64, CH], bf16, tag="expt",
                                        bufs=4)
                        nc.sync.dma_start(expt[:], d_exp[k, :, :])
                        # h1 and mps share a double-buffered PSUM tag
                        h1 = pps.tile([128, CH], f32, tag="hm", bufs=2)
                        for a in range(0, CH, 512):
                            b = a + 512
                            first = True
                            for (t, j0, j1) in ch["spans"]:
                                lo, hi = max(j0, a), min(j1, b)
                                if lo >= hi:
                                    continue
                                nc.tensor.matmul(h1[:, lo:hi], acomb[:, t, :],
                                                 expt[:, lo:hi], start=first,
                                                 stop=False,
                                                 skip_group_check=True)
                                first = False
                            nc.tensor.matmul(h1[:, a:b], wb[:],
                                             bg[:, 0, kk * CH + a:kk * CH + b],
                                             start=first, stop=True,
                                             skip_group_check=True)
                        r = esb.tile([128, CH], bf16, tag="r")
                        nc.scalar.activation(r[:, 0:512], h1[:, 0:512], AF.Relu)
                        nc.scalar.activation(r[:, 512:CH], h1[:, 512:CH],
                                             AF.Relu)
                        mps = pps.tile([128, CH], f32, tag="hm", bufs=2)
                        for a in range(0, CH, 512):
                            nc.tensor.matmul(mps[:, a:a + 512], w2[:],
                                             r[:, a:a + 512], start=True,
                                             stop=True)
                        mbf = esb.tile([128, CH], bf16, tag="mbf")
                        if k % 4 == 1:
                            nc.vector.tensor_scalar_mul(mbf[:], mps[:], 1.0)
                        else:
                            nc.scalar.activation(mbf[:, 0:512], mps[:, 0:512],
                                                 AF.Copy)
                            nc.scalar.activation(mbf[:, 512:CH], mps[:, 512:CH],
                                                 AF.Copy)
                        msq = esb.tile([128, CH], bf16, tag="msq")
                        if k % 2 == 1:
                            nc.vector.tensor_tensor(msq[:], mbf[:], mbf[:],
                                                    op=OP.mult)
                        else:
                            nc.scalar.activation(msq[:, 0:512], mps[:, 0:512],
                                                 AF.Square)
                            nc.scalar.activation(msq[:, 512:CH], mps[:, 512:CH],
                                                 AF.Square)
                        # sum/sumsq pre-reduction on PE (identity-matmul
                        # accumulation of window slices); min/max halved on DVE
                        # sum/sumsq pre-reduced on PE (bounded slice count),
                        # then one merged windowed reduce on DVE per run.
                        hs = phs.tile([128, 2, CH // 2], f32, tag="hs")
                        hmin = esb.tile([128, CH // 2], bf16, tag="hmin")
                        hmax = esb.tile([128, CH // 2], bf16, tag="hmax")
                        hpos = 0
                        qpos = 0
                        for (col0, ncols, d, D, eoff) in ch["runs"]:
                            h = D // 2
                            nq = 8 if D % 8 == 0 else (4 if D % 4 == 0 else 2)
                            q = D // nq
                            vm = mbf[:, eoff:eoff + ncols * D].rearrange(
                                "p (n D) -> p n D", D=D)
                            vq = msq[:, eoff:eoff + ncols * D].rearrange(
                                "p (n D) -> p n D", D=D)
                            for (row, vsrc) in ((0, vm), (1, vq)):
                                dst = hs[:, row, qpos:qpos + ncols * q]
                                for i in range(nq):
                                    nc.tensor.matmul(
                                        dst, identb[:],
                                        vsrc[:, :, i * q:(i + 1) * q],
                                        start=(i == 0), stop=(i == nq - 1),
                                        skip_group_check=True)
                            nc.vector.tensor_reduce(
                                s_ss[:, :, col0:col0 + ncols],
                                hs[:, :, qpos:qpos + ncols * q].rearrange(
                                    "p two (n q) -> p two n q", q=q),
                                axis=AX, op=OP.add)
                            def hv(buf):
                                return buf[:, hpos:hpos + ncols * h].rearrange(
                                    "p (n h) -> p n h", h=h)
                            nc.vector.tensor_tensor(
                                hv(hmin), vm[:, :, 0:h], vm[:, :, d - h:d],
                                op=OP.min)
                            nc.vector.tensor_tensor(
                                hv(hmax), vm[:, :, 0:h], vm[:, :, d - h:d],
                                op=OP.max)
                            nc.vector.tensor_reduce(
                                s_min[:, col0:col0 + ncols], hv(hmin),
                                axis=AX, op=OP.min)
                            nc.vector.tensor_reduce(
                                s_max[:, col0:col0 + ncols], hv(hmax),
                                axis=AX, op=OP.max)
                            hpos += ncols * h
                            qpos += ncols * q

                    if 1 <= k <= nstage:
                        load_stage(k - 1)

                    # ---- interleaved, pipelined post passes (3 stages)
                    for (stg, cc) in stage_plan.get(k, []):
                        pa_stage(stg, cc)

    nc.compile()
    return nc


# ----------------------------------------------------------------------------
# Entry point
# ----------------------------------------------------------------------------

_CACHE = {}
LAST_EXEC_NS = None


def kernel(**inputs):
    from concourse.bass_utils import run_bass_kernel_spmd

    atom_x = np.asarray(inputs["atom_x"], np.float32)
    bond_x = np.asarray(inputs["bond_x"]).astype(np.int64)
    edge_index = np.asarray(inputs["edge_index"])
    src = edge_index[0].astype(np.int64)

    p = make_plan(edge_index)
    w = pack_weights(inputs, p)

    xtok = np.zeros((128, NTOKP), BF16)
    xb = atom_x.astype(BF16)
    ii = np.arange(N)
    xtok_view = xtok.reshape(128, NTOKP // 128, 128)
    xtok_view[ii % 128, ii // 128, :] = xb

    in_maps = []
    for c in range(NCORES):
        srcv, bondv, arow = pack_core_edges(p, c, src, bond_x)
        exp = build_expdat(p, srcv, bondv, arow)
        sidx = build_srcidx(p, srcv)
        cols = p.col_node[c]
        xo = np.zeros((128, p.n_tot), np.float32)
        real = cols >= 0
        xo[:, real] = atom_x[cols[real]].T
        m = dict(expdat=exp, srcidx=sidx,
                 xbf=xo.astype(BF16), xfull=xtok)
        m.update({k: w[k] for k in (
            "wab", "wb", "ctab6", "w2", "w1post", "w2post", "wlin",
            "smallcols", "ones1", "identb", "biases", "invd_bc",
            "amp_bc", "att_bc", "corr")})
        in_maps.append(m)

    key = (p.n_tot, p.nch, str([c["runs"] for c in p.chunks]))
    if key not in _CACHE:
        _CACHE[key] = build_program(p)
    nc = _CACHE[key]

    res = run_bass_kernel_spmd(nc, in_maps, core_ids=list(range(NCORES)))
    global LAST_EXEC_NS
    LAST_EXEC_NS = res.exec_time_ns

    out = np.empty((N, H), np.float32)
    for c in range(NCORES):
        cols = p.col_node[c]
        real = cols >= 0
        out[cols[real]] = res.results[c]["out"].T[real]
    return out


# revision 40
# speedup vs baseline: 1.0741x; 1.0741x over previous
"""PNAConv message-passing kernel for 8 TRN2 NeuronCores.

Strategy (node-sharded, degree-grouped, engine-balanced):
  - Nodes sorted by in-degree globally, dealt round-robin to 8 cores; each
    core owns all edges of its nodes (no collectives).
  - Per-edge work in [128 feat, edges] layout:
      h1 = Acomb-onehot + wb @ gather(x_src); m = w2 @ relu(h1).
    Odd-degree segments are padded to even width with one extra slot whose
    one-hot selects a -1e9 row, so relu gives m == 0 exactly and every
    sum/sumsq window halves cleanly.  min/max windows use overlapped halving
    over the true (unpadded) degree, so the pad slot never pollutes them.
  - Engine split per chunk: ACT relu + square; Pool min/max halvings (+DGE
    for the gather); DVE sum/sq halvings + the 4 final windowed reduces.
  - Aggregates kept bf16; every post-MLP matmul is bf16 (1 cyc/row on PE).
  - LayerNorm tail is computed per 512-column chunk, pipelined behind the
    edge loop; output leaves as [128, n_tot] (host transposes).
"""

import sys
import numpy as np

sys.path.insert(0, "/opt/trn_rl_repo")

import ml_dtypes

N, E, H, T, FIN, DEGBINS = 20000, 320000, 128, 4, 32, 128
NCORES = 8
CH = 1024          # edge slots per compute chunk
GIDX = 4096        # indices per dma_gather call
EPS_LN = 1e-5
ATILE = 58         # node columns per Acomb tile (+1 neg +5 bond = 64 rows)
NEGROW = 58
BONDROW = 59
NTOKP = 20096      # node table padded to 157*128

BF16 = ml_dtypes.bfloat16


# ----------------------------------------------------------------------------
# Planning: uniform per-core structure derived from the degree profile
# ----------------------------------------------------------------------------

class Plan:
    pass


def slotw(d):
    """Slot width of a degree-d segment (odd degrees pad by one)."""
    return d + (d & 1)


def make_plan(edge_index):
    src = np.asarray(edge_index[0]).astype(np.int64)
    dst = np.asarray(edge_index[1]).astype(np.int64)
    deg = np.bincount(dst, minlength=N)

    order = np.argsort(deg, kind="stable")
    core_nodes = [order[c::NCORES] for c in range(NCORES)]

    degrees = [int(d) for d in np.unique(deg)]
    n_d = {}
    for d in degrees:
        n_d[d] = max(int((deg[cn] == d).sum()) for cn in core_nodes)

    n_tot = sum(n_d.values())
    pad0 = (-n_tot) % 128
    if pad0:
        n_d[0] = n_d.get(0, 0) + pad0
        n_tot += pad0
    dlist = sorted(n_d.keys())

    col_deg = np.concatenate([np.full(n_d[d], d, np.int64) for d in dlist])
    assert len(col_deg) == n_tot and n_tot % 128 == 0
    col_D = col_deg + (col_deg & 1)

    # chunking (identical across cores); runs carry (col0, ncols, d, D, eoff)
    chunks = []
    c = 0
    while c < n_tot:
        e_used, c0 = 0, c
        while c < n_tot and e_used + col_D[c] <= CH:
            e_used += col_D[c]
            c += 1
        assert c > c0, "degree larger than chunk size"
        runs, eoff, rc = [], 0, c0
        while rc < c:
            d = int(col_deg[rc])
            rn = rc
            while rn < c and col_deg[rn] == d:
                rn += 1
            D = slotw(d)
            if d > 0:
                runs.append((int(rc), int(rn - rc), d, D, int(eoff)))
            eoff += (rn - rc) * D
            rc = rn
        assert c - c0 <= 256, "chunk column count exceeds hs tile width"
        chunks.append(dict(c0=int(c0), c1=int(c), nslot=int(e_used), runs=runs))
    while len(chunks) % (GIDX // CH):
        chunks.append(dict(c0=n_tot, c1=n_tot, nslot=0, runs=[]))
    nch = len(chunks)

    # A-tile matmul spans per chunk: (tile, j0, j1) over the chunk's CH slots
    for ch in chunks:
        spans, j = [], 0
        for col in range(ch["c0"], ch["c1"]):
            t, D = col // ATILE, int(col_D[col])
            if D == 0:
                continue
            if spans and spans[-1][0] == t:
                spans[-1][2] += D
            else:
                spans.append([t, j, j + D])
            j += D
        if spans:
            spans[-1][2] = CH          # cover tail pad slots (one-hot zero)
        ch["spans"] = [(int(a), int(b), int(cc)) for a, b, cc in spans]

    p = Plan()
    p.deg, p.core_nodes = deg, core_nodes
    p.n_d, p.dlist, p.n_tot = n_d, dlist, n_tot
    p.col_deg, p.col_D = col_deg, col_D
    p.chunks, p.nch = chunks, nch
    p.zero_cols = int(n_d.get(0, 0))
    p.ntiles = (n_tot + ATILE - 1) // ATILE

    # which gather groups are fully dead (trailing pad chunks)
    gpc = GIDX // CH
    p.group_live = [any(chunks[g * gpc + i]["runs"] for i in range(gpc))
                    for g in range(nch // gpc)]

    # per-core column -> global node id (-1 for dummy)
    p.col_node = []
    for cn in core_nodes:
        cols = np.full(n_tot, -1, np.int64)
        off = 0
        for d in dlist:
            mine = cn[deg[cn] == d]
            cols[off:off + len(mine)] = mine
            off += n_d[d]
        p.col_node.append(cols)

    eo = np.argsort(dst, kind="stable")
    starts = np.zeros(N + 1, np.int64)
    np.cumsum(np.bincount(dst, minlength=N), out=starts[1:])
    p.edge_order, p.edge_starts = eo, starts
    p.src = src
    return p


def pack_core_edges(p, c, src, bond):
    """Per-core padded edge arrays: src ids, bond ids, one-hot row kinds.

    arow: 0..ATILE-1 node row, NEGROW for in-run pad slots, -1 for dead slots.
    """
    EP = p.nch * CH
    srcv = np.zeros(EP, np.int64)
    bondv = np.full(EP, -1, np.int64)
    arow = np.full(EP, -1, np.int64)
    cols = p.col_node[c]
    eo, starts = p.edge_order, p.edge_starts
    for k, ch in enumerate(p.chunks):
        j = k * CH
        for col in range(ch["c0"], ch["c1"]):
            d = int(p.col_deg[col])
            D = int(p.col_D[col])
            if D == 0:
                continue
            node = cols[col]
            if node >= 0 and d > 0:
                eidx = eo[starts[node]:starts[node + 1]]
                srcv[j:j + d] = src[eidx]
                bondv[j:j + d] = bond[eidx]
                arow[j:j + d] = col % ATILE
            if D > d:
                arow[j + d:j + D] = NEGROW
            j += D
    return srcv, bondv, arow


def build_expdat(p, srcv, bondv, arow):
    EP = p.nch * CH
    exp = np.zeros((p.nch, 64, CH), BF16)
    k = np.arange(EP) // CH
    j = np.arange(EP) % CH
    real = (arow >= 0) & (arow < NEGROW)
    exp[k[real], arow[real], j[real]] = 1
    exp[k[real], BONDROW + bondv[real], j[real]] = 1
    padm = arow == NEGROW
    exp[k[padm], NEGROW, j[padm]] = 1
    return exp


def build_srcidx(p, srcv):
    """[ngrp, 128, GIDX//16]; one gather call per group of GIDX edges."""
    ngrp = (p.nch * CH) // GIDX
    out = np.zeros((ngrp, 128, GIDX // 16), np.int16)
    i = np.arange(GIDX)
    v = srcv.reshape(ngrp, GIDX).astype(np.int16)
    for rep in range(8):
        out[:, rep * 16 + (i % 16), i // 16] = v[:, i]
    return out


# ----------------------------------------------------------------------------
# Weight packing (host)
# ----------------------------------------------------------------------------

def blockdiag(ws):
    ws = np.asarray(ws)
    t, f, g = ws.shape
    out = np.zeros((t * f, t * g), np.float32)
    for i in range(t):
        out[i * f:(i + 1) * f, i * g:(i + 1) * g] = ws[i]
    return out


def pack_weights(ins, p):
    w = {}
    pre_w1 = np.asarray(ins["pre_w1"], np.float32)      # [T, 96, 32]
    w["wab"] = blockdiag(pre_w1[:, 0:FIN]).astype(BF16)
    w["wb"] = blockdiag(pre_w1[:, FIN:2 * FIN]).astype(BF16)
    table5 = (np.asarray(ins["bond_emb"], np.float32) @ np.asarray(ins["enc_w"], np.float32)
              + np.asarray(ins["enc_b"], np.float32))                       # [5, 32]
    wc = pre_w1[:, 2 * FIN:3 * FIN]                                        # [T, 32, 32]
    ctab = np.zeros((5, H), np.float32)
    b1 = np.asarray(ins["pre_b1"], np.float32)                             # [T, 32]
    for k in range(5):
        for t in range(T):
            ctab[k, t * FIN:(t + 1) * FIN] = table5[k] @ wc[t] + b1[t]
    ctab6 = np.concatenate([np.full((1, H), -1.0e9, np.float32), ctab])
    w["ctab6"] = np.ascontiguousarray(
        np.tile(ctab6.astype(BF16), (1, p.ntiles)))                         # [6, ntiles*128]
    w["w2"] = blockdiag(np.asarray(ins["pre_w2"], np.float32)).astype(BF16)

    post_w1 = np.asarray(ins["post_w1"], np.float32)                       # [T, 512, 32]
    blocks = []
    rowsel = [0] + [1 + a for a in range(5)] + [6 + a for a in range(5)] + [11 + a for a in range(5)]
    for r in rowsel:
        blocks.append(blockdiag(post_w1[:, 32 * r:32 * (r + 1)]))
    w["w1post"] = np.ascontiguousarray(
        np.concatenate(blocks, axis=1)).astype(BF16)                        # [128, 16*128]
    w["w2post"] = blockdiag(np.asarray(ins["post_w2"], np.float32)).astype(BF16)
    lin_w = np.asarray(ins["lin_w"], np.float32)
    w["wlin"] = lin_w.astype(BF16)
    lin_b = np.asarray(ins["lin_b"], np.float32)

    # small column pack: [onescol, wlinsum] as [128, 2] bf16
    small = np.zeros((128, 2), np.float32)
    small[:, 0] = 1.0
    small[:, 1] = lin_w.sum(axis=1)
    w["smallcols"] = small.astype(BF16)
    w["ones1"] = np.ones((1, 128), BF16)
    w["identb"] = np.eye(128, dtype=BF16)

    biases = np.zeros((128, 6), np.float32)
    biases[:, 0] = np.asarray(ins["post_b1"], np.float32).reshape(-1)
    biases[:, 1] = np.asarray(ins["post_b2"], np.float32).reshape(-1)
    biases[:, 2] = lin_b
    biases[:, 3] = np.asarray(ins["ln_g"], np.float32)
    biases[:, 4] = np.asarray(ins["ln_b"], np.float32)
    biases[:, 5] = -float(lin_b.sum()) / 128.0
    w["biases"] = biases

    # per-node scaler rows and the pre_b2 correction tensor
    mol_deg = np.asarray(ins["mol_deg"], np.float64)
    bins = np.arange(DEGBINS, dtype=np.float64)
    avg_log = float((np.log(bins + 1.0) * mol_deg).sum() / mol_deg.sum())

    d = p.col_deg.astype(np.float64)
    d1 = np.maximum(d, 1.0)
    logd = np.log(d1 + 1.0)
    invd = (1.0 / d1).astype(np.float32)
    amp = (logd / avg_log).astype(np.float32)
    att = (avg_log / logd).astype(np.float32)
    w["invd_bc"] = np.ascontiguousarray(np.broadcast_to(invd, (128, p.n_tot))).astype(BF16)
    w["amp_bc"] = np.ascontiguousarray(np.broadcast_to(amp, (128, p.n_tot))).astype(BF16)
    w["att_bc"] = np.ascontiguousarray(np.broadcast_to(att, (128, p.n_tot))).astype(BF16)

    b2 = np.asarray(ins["pre_b2"], np.float32).reshape(-1)                  # [128]
    # correction for reducing m without its bias; folded through post-MLP L1.
    WT = w["w1post"].astype(np.float32).reshape(128, 16, 128)
    def grp(i0):
        csum = WT[:, i0, :].T @ b2
        cmmm = sum(WT[:, i0 + a, :].T @ b2 for a in (1, 2, 3))
        return csum, cmmm
    cs_id, cm_id = grp(1)
    cs_am, cm_am = grp(6)
    cs_at, cm_at = grp(11)
    dcol = d.astype(np.float32)
    corr = (cm_id[:, None] + np.outer(cs_id, dcol)
            + amp[None, :] * (cm_am[:, None] + np.outer(cs_am, dcol))
            + att[None, :] * (cm_at[:, None] + np.outer(cs_at, dcol)))
    w["corr"] = np.ascontiguousarray(corr).astype(BF16)
    return w


# ----------------------------------------------------------------------------
# Device program
# ----------------------------------------------------------------------------

def build_program(p):
    import concourse.bass as bass
    import concourse.tile as tile
    from concourse import bacc, mybir

    f32 = mybir.dt.float32
    bf16 = mybir.dt.bfloat16
    i16 = mybir.dt.int16
    AX = mybir.AxisListType.X
    OP = mybir.AluOpType
    AF = mybir.ActivationFunctionType

    n_tot, nch, ntiles = p.n_tot, p.nch, p.ntiles

    nc = bacc.Bacc("TRN2", target_bir_lowering=False, debug=False)

    d_exp = nc.dram_tensor("expdat", [nch, 64, CH], bf16, kind="ExternalInput")
    ngrp = (nch * CH) // GIDX
    d_idx = nc.dram_tensor("srcidx", [ngrp, 128, GIDX // 16], i16,
                           kind="ExternalInput")
    d_xbf = nc.dram_tensor("xbf", [128, n_tot], bf16, kind="ExternalInput")
    d_xfull = nc.dram_tensor("xfull", [128, NTOKP], bf16, kind="ExternalInput")
    d_wab = nc.dram_tensor("wab", [128, 128], bf16, kind="ExternalInput")
    d_wb = nc.dram_tensor("wb", [128, 128], bf16, kind="ExternalInput")
    d_ctab6 = nc.dram_tensor("ctab6", [6, ntiles * 128], bf16, kind="ExternalInput")
    d_w2 = nc.dram_tensor("w2", [128, 128], bf16, kind="ExternalInput")
    d_w1post = nc.dram_tensor("w1post", [128, 16 * 128], bf16, kind="ExternalInput")
    d_w2post = nc.dram_tensor("w2post", [128, 128], bf16, kind="ExternalInput")
    d_wlin = nc.dram_tensor("wlin", [128, 128], bf16, kind="ExternalInput")
    d_small = nc.dram_tensor("smallcols", [128, 2], bf16, kind="ExternalInput")
    d_ones1 = nc.dram_tensor("ones1", [1, 128], bf16, kind="ExternalInput")
    d_identb = nc.dram_tensor("identb", [128, 128], bf16, kind="ExternalInput")
    d_bias = nc.dram_tensor("biases", [128, 6], f32, kind="ExternalInput")
    d_invd = nc.dram_tensor("invd_bc", [128, n_tot], bf16, kind="ExternalInput")
    d_amp = nc.dram_tensor("amp_bc", [128, n_tot], bf16, kind="ExternalInput")
    d_att = nc.dram_tensor("att_bc", [128, n_tot], bf16, kind="ExternalInput")
    d_corr = nc.dram_tensor("corr", [128, n_tot], bf16, kind="ExternalInput")
    d_out = nc.dram_tensor("out", [128, n_tot], f32, kind="ExternalOutput")

    # col-chunk cc (512 cols) is ready after the last edge chunk touching it
    ready = {}
    for cc in range(n_tot // 512):
        last = 0
        for k2, ch2 in enumerate(p.chunks):
            if ch2["c0"] < 512 * (cc + 1) and ch2["runs"]:
                last = k2
        ready.setdefault(last, []).append(cc)
    first_ready = min(ready.keys())

    with nc.allow_low_precision(reason="bf16 aggregates within rel-err budget"):
      with tile.TileContext(nc) as tc:
        with tc.tile_pool(name="persist", bufs=1) as per:
            # --- persistent SBUF tensors; critical loads first ---------------
            xbf = per.tile([128, n_tot], bf16)
            nc.sync.dma_start(xbf[:], d_xbf[:])
            wab = per.tile([128, 128], bf16); nc.sync.dma_start(wab[:], d_wab[:])
            wb = per.tile([128, 128], bf16); nc.sync.dma_start(wb[:], d_wb[:])
            w2 = per.tile([128, 128], bf16); nc.sync.dma_start(w2[:], d_w2[:])
            identb = per.tile([128, 128], bf16)
            nc.sync.dma_start(identb[:], d_identb[:])
            xtok = per.tile([128, NTOKP], bf16)
            nc.sync.dma_start(xtok[:], d_xfull[:])

            acomb = per.tile([64, ntiles, 128], bf16)
            w1post = per.tile([128, 16, 128], bf16)
            w2post = per.tile([128, 128], bf16)
            wlin = per.tile([128, 128], bf16)
            smallcols = per.tile([128, 2], bf16)
            ones1 = per.tile([1, 128], bf16)
            biases = per.tile([128, 6], f32)
            invd_bc = per.tile([128, n_tot], bf16)
            amp_bc = per.tile([128, n_tot], bf16)
            att_bc = per.tile([128, n_tot], bf16)
            corr = per.tile([128, n_tot], bf16)
            eps_col = per.tile([128, 1], f32)

            def load_stage(st):
                if st == 0:
                    nc.sync.dma_start(w1post[:].rearrange("p a b -> p (a b)"),
                                      d_w1post[:])
                    nc.sync.dma_start(biases[:], d_bias[:])
                    nc.gpsimd.memset(eps_col[:], EPS_LN)
                elif st == 1:
                    nc.sync.dma_start(w2post[:], d_w2post[:])
                    nc.sync.dma_start(wlin[:], d_wlin[:])
                    nc.sync.dma_start(smallcols[:], d_small[:])
                    nc.sync.dma_start(ones1[:], d_ones1[:])
            nstage = 2

            # aggregate buffers (bf16)
            s_ss = per.tile([128, 2, n_tot], bf16)
            s_min = per.tile([128, n_tot], bf16)
            s_max = per.tile([128, n_tot], bf16)

            # --- setup: Acomb (bf16 matmuls; copies on Pool) -----------------
            nc.gpsimd.memset(acomb[:], 0.0)
            nc.sync.dma_start(
                acomb[NEGROW:64, :, :],
                d_ctab6[:].rearrange("p (t f) -> p t f", f=128))
            with tc.tile_pool(name="su_ps", bufs=2, space="PSUM") as sps:
                for t in range(ntiles):
                    nt = min(ATILE, n_tot - t * ATILE)
                    aps = sps.tile([64, 128], f32, tag="aps")
                    nc.tensor.matmul(aps[0:nt, :], xbf[:, t * ATILE:t * ATILE + nt],
                                     wab[:], start=True, stop=True)
                    nc.vector.tensor_scalar_mul(acomb[0:nt, t, :],
                                                aps[0:nt, :], 1.0)

            if p.zero_cols:
                z = p.zero_cols
                nc.gpsimd.memset(s_ss[:, :, 0:z], 0.0)
                nc.gpsimd.memset(s_min[:, 0:z], 0.0)
                nc.gpsimd.memset(s_max[:, 0:z], 0.0)

            # --- edge phase with interleaved post passes ---------------------
            with (
                tc.tile_pool(name="eg_ps", bufs=1, space="PSUM") as pps,
                tc.tile_pool(name="eg_hs", bufs=1, space="PSUM") as phs,
                tc.tile_pool(name="pa_uid", bufs=1, space="PSUM") as puid,
                tc.tile_pool(name="pa_acc", bufs=1, space="PSUM") as pacc,
                tc.tile_pool(name="eg_sb", bufs=2) as esb,
                tc.tile_pool(name="eg_gt", bufs=2) as egt,
                tc.tile_pool(name="pa_sb", bufs=2) as pasb,
            ):
                # stage s of col-chunk cc issues at chunk ready[cc]+s; a
                # 5-deep pipeline so no engine convoys behind a full passA.
                # stage 0 = scaler-table slice prefetch (can fire early).
                stage_plan = {}
                for k0, ccs in ready.items():
                    for cc in ccs:
                        stage_plan.setdefault(max(0, k0 - 1), []).append((0, cc))
                        for s in range(1, 5):
                            stage_plan.setdefault(min(k0 + s, nch - 1),
                                                  []).append((s, cc))
                # re-sort so stages issue in (chunk, stage) order
                stage_plan = {k2: sorted(v) for k2, v in stage_plan.items()}
                cst = {}

                def pa_stage(stg, cc):
                    a, b = 512 * cc, 512 * (cc + 1)
                    s_sum = s_ss[:, 0, a:b]
                    s_sumsq = s_ss[:, 1, a:b]
                    st = cst.setdefault(cc, {})
                    if stg == 0:
                        # prefetch this block's scaler-table slices
                        nc.sync.dma_start(invd_bc[:, a:b], d_invd[:, a:b])
                        nc.sync.dma_start(corr[:, a:b], d_corr[:, a:b])
                        nc.sync.dma_start(amp_bc[:, a:b], d_amp[:, a:b])
                        nc.sync.dma_start(att_bc[:, a:b], d_att[:, a:b])
                    elif stg == 1:
                        # statistics (all bf16)
                        s_mean = pasb.tile([128, 512], bf16, tag="tmean")
                        nc.vector.tensor_tensor(s_mean[:], s_sum,
                                                invd_bc[:, a:b], op=OP.mult)
                        m2 = pasb.tile([128, 512], bf16, tag="tb")
                        nc.vector.tensor_tensor(m2[:], s_mean[:],
                                                s_mean[:], op=OP.mult)
                        var = pasb.tile([128, 512], bf16, tag="ta")
                        nc.vector.tensor_tensor(var[:], s_sumsq,
                                                invd_bc[:, a:b], op=OP.mult)
                        nc.vector.tensor_tensor(var[:], var[:], m2[:],
                                                op=OP.subtract)
                        vclamp = pasb.tile([128, 512], bf16, tag="tvc")
                        nc.vector.tensor_scalar_max(vclamp[:], var[:], 0.0)
                        tstd = pasb.tile([128, 512], bf16, tag="tstd")
                        nc.scalar.activation(tstd[:], vclamp[:], AF.Sqrt,
                                             bias=eps_col[:])
                        st["s_mean"], st["tstd"] = s_mean, tstd
                    elif stg == 2:
                        s_mean, tstd = st["s_mean"], st["tstd"]
                        uid = puid.tile([128, 512], f32, tag="uid")
                        nc.tensor.matmul(uid[:], w1post[:, 0, :], xbf[:, a:b],
                                         start=True, stop=False)
                        nc.tensor.matmul(uid[:], w1post[:, 1, :], s_sum,
                                         start=False, stop=False)
                        nc.tensor.matmul(uid[:], w1post[:, 2, :], s_mean[:],
                                         start=False, stop=False)
                        nc.tensor.matmul(uid[:], w1post[:, 3, :], s_min[:, a:b],
                                         start=False, stop=False)
                        nc.tensor.matmul(uid[:], w1post[:, 4, :], s_max[:, a:b],
                                         start=False, stop=False)
                        nc.tensor.matmul(uid[:], w1post[:, 5, :], tstd[:],
                                         start=False, stop=False)
                        nc.tensor.matmul(uid[:], identb[:], corr[:, a:b],
                                         start=False, stop=False)
                        uam = pacc.tile([128, 512], f32, tag="uacc")
                        nc.tensor.matmul(uam[:], w1post[:, 6, :], s_sum,
                                         start=True, stop=False)
                        nc.tensor.matmul(uam[:], w1post[:, 7, :], s_mean[:],
                                         start=False, stop=False)
                        nc.tensor.matmul(uam[:], w1post[:, 8, :], s_min[:, a:b],
                                         start=False, stop=False)
                        nc.tensor.matmul(uam[:], w1post[:, 9, :], s_max[:, a:b],
                                         start=False, stop=False)
                        nc.tensor.matmul(uam[:], w1post[:, 10, :], tstd[:],
                                         start=False, stop=True)
                        st["uid"], st["uam"] = uid, uam
                    elif stg == 3:
                        s_mean, tstd, uid = st["s_mean"], st["tstd"], st["uid"]
                        t1 = pasb.tile([128, 512], bf16, tag="ta")
                        nc.vector.tensor_tensor(t1[:], st["uam"][:],
                                                amp_bc[:, a:b], op=OP.mult)
                        nc.tensor.matmul(uid[:], identb[:], t1[:],
                                         start=False, stop=False)
                        uat = pacc.tile([128, 512], f32, tag="uacc")
                        nc.tensor.matmul(uat[:], w1post[:, 11, :], s_sum,
                                         start=True, stop=False)
                        nc.tensor.matmul(uat[:], w1post[:, 12, :], s_mean[:],
                                         start=False, stop=False)
                        nc.tensor.matmul(uat[:], w1post[:, 13, :], s_min[:, a:b],
                                         start=False, stop=False)
                        nc.tensor.matmul(uat[:], w1post[:, 14, :], s_max[:, a:b],
                                         start=False, stop=False)
                        nc.tensor.matmul(uat[:], w1post[:, 15, :], tstd[:],
                                         start=False, stop=True)
                        t2 = pasb.tile([128, 512], bf16, tag="tb")
                        nc.vector.tensor_tensor(t2[:], uat[:], att_bc[:, a:b],
                                                op=OP.mult)
                        nc.tensor.matmul(uid[:], identb[:], t2[:],
                                         start=False, stop=True)
                        r1 = pasb.tile([128, 512], bf16, tag="tr")
                        nc.scalar.activation(r1[:], uid[:], AF.Relu,
                                             bias=biases[:, 0:1])
                        u2 = pacc.tile([128, 512], f32, tag="uacc")
                        nc.tensor.matmul(u2[:], w2post[:], r1[:],
                                         start=True, stop=True)
                        s2 = pasb.tile([128, 512], bf16, tag="ta")
                        nc.scalar.activation(s2[:], u2[:], AF.Identity,
                                             bias=biases[:, 1:2])
                        st["s2"] = s2
                    else:
                        s2 = st["s2"]
                        # final linear with the LN mean-subtract folded in
                        lp = pacc.tile([128, 512], f32, tag="uacc")
                        nc.tensor.matmul(lp[:], wlin[:], s2[:],
                                         start=True, stop=False)
                        mu_p = puid.tile([1, 512], f32, tag="uid")
                        nc.tensor.matmul(mu_p[:], smallcols[:, 1:2], s2[:],
                                         start=True, stop=True)
                        muN = pasb.tile([1, 512], bf16, tag="muN")
                        nc.scalar.activation(muN[:], mu_p[:], AF.Identity,
                                             scale=-1.0 / 128.0,
                                             bias=biases[0:1, 5:6])
                        nc.tensor.matmul(lp[:], ones1[:], muN[:],
                                         start=False, stop=True)
                        lbuf = pasb.tile([128, 512], bf16, tag="tb")
                        nc.scalar.activation(lbuf[:], lp[:], AF.Identity,
                                             bias=biases[:, 2:3])
                        lsq = pasb.tile([128, 512], bf16, tag="tvc")
                        nc.vector.tensor_tensor(lsq[:], lbuf[:],
                                                lbuf[:], op=OP.mult)
                        sq_p = puid.tile([1, 512], f32, tag="uid")
                        nc.tensor.matmul(sq_p[:], smallcols[:, 0:1], lsq[:],
                                         start=True, stop=True)
                        rsd = pasb.tile([1, 512], f32, tag="rsd")
                        nc.scalar.activation(rsd[:], sq_p[:], AF.Sqrt,
                                             scale=1.0 / 128.0,
                                             bias=eps_col[0:1, :])
                        rs = pasb.tile([1, 512], bf16, tag="muN")
                        nc.vector.reciprocal(rs[:], rsd[:])
                        rsb = pacc.tile([128, 512], f32, tag="uacc")
                        nc.tensor.matmul(rsb[:], ones1[:], rs[:],
                                         start=True, stop=True)
                        tno = pasb.tile([128, 512], bf16, tag="ta")
                        nc.vector.tensor_tensor(tno[:], lbuf[:], rsb[:],
                                                op=OP.mult)
                        fin = pasb.tile([128, 512], bf16, tag="tr")
                        nc.scalar.activation(fin[:], tno[:], AF.Relu,
                                             bias=biases[:, 4:5],
                                             scale=biases[:, 3:4])
                        outb = pasb.tile([128, 512], f32, tag="outb")
                        nc.vector.tensor_tensor(outb[:], fin[:],
                                                xbf[:, a:b], op=OP.add)
                        nc.sync.dma_start(d_out[:, a:b], outb[:])
                        cst.pop(cc, None)

                for k in range(nch):
                    kk = k % (GIDX // CH)
                    g = k // (GIDX // CH)
                    if kk == 0 and p.group_live[g]:
                        gidx = egt.tile([128, GIDX // 16], i16, tag="gidx")
                        nc.sync.dma_start(gidx[:], d_idx[g, :, :])
                        bg = egt.tile([128, 1, GIDX], bf16, tag="bg")
                        nc.gpsimd.dma_gather(
                            bg[:], xtok[:], gidx[:], GIDX, GIDX, 128,
                            transpose=True, sbuf_tokens_per_rank=128,
                            sbuf_free_dim_per_rank=256, single_packet=False,
                        )
                    ch = p.chunks[k]
                    if ch["runs"]:
                        expt = esb.tile([64, CH], bf16, tag="expt")
                        nc.sync.dma_start(expt[:], d_exp[k, :, :])
                        # h1 and mps share a double-buffered PSUM tag
                        h1 = pps.tile([128, CH], f32, tag="hm", bufs=2)
                        for a in range(0, CH, 512):
                            b = a + 512
                            first = True
                            for (t, j0, j1) in ch["spans"]:
                                lo, hi = max(j0, a), min(j1, b)
                                if lo >= hi:
                                    continue
                                nc.tensor.matmul(h1[:, lo:hi], acomb[:, t, :],
                                                 expt[:, lo:hi], start=first,
                                                 stop=False,
                                                 skip_group_check=True)
                                first = False
                            nc.tensor.matmul(h1[:, a:b], wb[:],
                                             bg[:, 0, kk * CH + a:kk * CH + b],
                                             start=first, stop=True,
                                             skip_group_check=True)
                        r = esb.tile([128, CH], bf16, tag="r")
                        nc.scalar.activation(r[:], h1[:], AF.Relu)
                        mps = pps.tile([128, CH], f32, tag="hm", bufs=2)
                        for a in range(0, CH, 512):
                            nc.tensor.matmul(mps[:, a:a + 512], w2[:],
                                             r[:, a:a + 512], start=True,
                                             stop=True)
                        mbf = esb.tile([128, CH], bf16, tag="mbf")
                        nc.scalar.activation(mbf[:], mps[:], AF.Copy)
                        msq = esb.tile([128, CH], bf16, tag="msq")
                        if k % 2 == 1:
                            nc.vector.tensor_tensor(msq[:], mbf[:], mbf[:],
                                                    op=OP.mult)
                        else:
                            nc.scalar.activation(msq[:], mps[:], AF.Square)
                        # sum/sumsq pre-reduction on PE (identity-matmul
                        # accumulation of window slices); min/max halved on DVE
                        # segment sum/sumsq fully on PE: d accumulating
                        # identity matmuls per run collapse each window to a
                        # single f32 column in PSUM.
                        nc0 = ch["c0"]
                        qtot = ch["c1"] - nc0
                        hs = phs.tile([128, 2, 256], f32, tag="hs")
                        hmin = esb.tile([128, CH // 2], bf16, tag="hmin")
                        hmax = esb.tile([128, CH // 2], bf16, tag="hmax")
                        hpos = 0
                        for (col0, ncols, d, D, eoff) in ch["runs"]:
                            h = D // 2
                            vm = mbf[:, eoff:eoff + ncols * D].rearrange(
                                "p (n D) -> p n D", D=D)
                            vq = msq[:, eoff:eoff + ncols * D].rearrange(
                                "p (n D) -> p n D", D=D)
                            qp = col0 - nc0
                            for (row, vsrc) in ((0, vm), (1, vq)):
                                dst = hs[:, row, qp:qp + ncols]
                                for i in range(d):
                                    nc.tensor.matmul(
                                        dst, identb[:], vsrc[:, :, i],
                                        start=(i == 0), stop=(i == d - 1),
                                        skip_group_check=True)
                            def hv(buf):
                                return buf[:, hpos:hpos + ncols * h].rearrange(
                                    "p (n h) -> p n h", h=h)
                            nc.vector.tensor_tensor(
                                hv(hmin), vm[:, :, 0:h], vm[:, :, d - h:d],
                                op=OP.min)
                            nc.vector.tensor_tensor(
                                hv(hmax), vm[:, :, 0:h], vm[:, :, d - h:d],
                                op=OP.max)
                            nc.vector.tensor_reduce(
                                s_min[:, col0:col0 + ncols], hv(hmin),
                                axis=AX, op=OP.min)
                            nc.vector.tensor_reduce(
                                s_max[:, col0:col0 + ncols], hv(hmax),
                                axis=AX, op=OP.max)
                            hpos += ncols * h
                        nc.vector.tensor_scalar_mul(
                            s_ss[:, :, nc0:nc0 + qtot], hs[:, :, 0:qtot], 1.0)

                    if 1 <= k <= nstage:
                        load_stage(k - 1)

                    # ---- interleaved, pipelined post passes (3 stages)
                    for (stg, cc) in stage_plan.get(k, []):
                        pa_stage(stg, cc)

    nc.compile()
    return nc


# ----------------------------------------------------------------------------
# Entry point
# ----------------------------------------------------------------------------

_CACHE = {}
LAST_EXEC_NS = None


def kernel(**inputs):
    from concourse.bass_utils import run_bass_kernel_spmd

    atom_x = np.asarray(inputs["atom_x"], np.float32)
    bond_x = np.asarray(inputs["bond_x"]).astype(np.int64)
    edge_index = np.asarray(inputs["edge_index"])
    src = edge_index[0].astype(np.int64)

    p = make_plan(edge_index)
    w = pack_weights(inputs, p)

    xtok = np.zeros((128, NTOKP), BF16)
    xb = atom_x.astype(BF16)
    ii = np.arange(N)
    xtok_view = xtok.reshape(128, NTOKP // 128, 128)
    xtok_view[ii % 128, ii // 128, :] = xb

    in_maps = []
    for c in range(NCORES):
        srcv, bondv, arow = pack_core_edges(p, c, src, bond_x)
        exp = build_expdat(p, srcv, bondv, arow)
        sidx = build_srcidx(p, srcv)
        cols = p.col_node[c]
        xo = np.zeros((128, p.n_tot), np.float32)
        real = cols >= 0
        xo[:, real] = atom_x[cols[real]].T
        m = dict(expdat=exp, srcidx=sidx,
                 xbf=xo.astype(BF16), xfull=xtok)
        m.update({k: w[k] for k in (
            "wab", "wb", "ctab6", "w2", "w1post", "w2post", "wlin",
            "smallcols", "ones1", "identb", "biases", "invd_bc",
            "amp_bc", "att_bc", "corr")})
        in_maps.append(m)

    key = (p.n_tot, p.nch, str([c["runs"] for c in p.chunks]))
    if key not in _CACHE:
        _CACHE[key] = build_program(p)
    nc = _CACHE[key]

    res = run_bass_kernel_spmd(nc, in_maps, core_ids=list(range(NCORES)))
    global LAST_EXEC_NS
    LAST_EXEC_NS = res.exec_time_ns

    out = np.empty((N, H), np.float32)
    for c in range(NCORES):
        cols = p.col_node[c]
        real = cols >= 0
        out[cols[real]] = res.results[c]["out"].T[real]
    return out
